# revision 19
# baseline (speedup 1.0000x reference)
"""AttentiveGraphConvolution (GAT-style layer) on 8 trn2 NeuronCores.

Math (reference):
    h   = x @ W                       [N, D]
    a_s = h @ attn_self               [N, 1]
    a_n = h @ attn_neigh              [N, 1]
    e   = leaky_relu(a_s + a_n.T, 0.2)
    e   = e + NEG_INF * (1 - adj)
    out = relu(softmax(e, -1) @ h)

Reformulation (exact up to rounding): with exp(leaky(s)) = max(e^s, e^{0.2 s}),

    exp(e_ij) = e^{0.2 a_s_i} * e^{a_n_j} * max(e^{0.8 a_s_i}, e^{-0.8 a_n_j})

The row factor e^{0.2 a_s_i} cancels in the softmax.  With
    wb_i = e^{0.8 a_s_i}      (per output node, broadcast tile)
    c_j  = e^{-0.8 a_n_j}     (per source node, per-partition scalar)
    ew_j = e^{a_n_j}          (row-sum weights)
    hw_j = ew_j * h_j         (pre-scaled h rows)
one DVE op per adjacency chunk produces q_ji = max(wb_i, c_j) * adjT_ji and

    out_i = relu( (sum_j hw_j q_ji) / (sum_j ew_j q_ji) )

Sharding: output rows i across 8 cores; each core streams its adjT slab
[n, s] (bf16, binary-exact).  A single AllGather moves the combined
[hw | ew | a_n] payload (~270 KB/core bf16); everything else is local.
"""

import numpy as np

N = 8192
DIN = 512
DOUT = 128
NCORES = 8
S = N // NCORES     # 1024 output rows per core
GP = 8              # adjacency chunks per super-chunk (one DMA each)
PW = 132            # payload row width: 128 hw | ew | a_n | 2 pad

PLAN_B = True      # True: q-as-weights matmuls (num+den in one pass)
ADJ_FP8 = False     # True: stream adjacency as fp8e4 (binary-exact)


def _emit(nc, tc, ctx, n, s, din, dout):
    from concourse import masks, mybir

    f32 = mybir.dt.float32
    f32r = mybir.dt.float32r
    bf16 = mybir.dt.bfloat16
    fp8 = mybir.dt.float8e4
    adt = fp8 if ADJ_FP8 else bf16
    AF = mybir.ActivationFunctionType
    ALU = mybir.AluOpType

    P = 128
    jc_n = n // P           # 64 source-node chunks
    sc_n = s // P           # 8 chunks in the local row slab
    kc_n = din // P         # 4 contraction chunks for x @ W
    nb = 512                # matmul moving-dim block
    ib_n = s // nb          # 2 i blocks per core
    g_n = jc_n // GP        # 8 adj super-chunks

    adjt = nc.dram_tensor("adjt", [n, s], adt, kind="ExternalInput")
    xt = nc.dram_tensor("xt", [din, s], bf16, kind="ExternalInput")
    wmat = nc.dram_tensor("wmat", [din, dout], bf16, kind="ExternalInput")
    wtr = nc.dram_tensor("wt", [dout, din], bf16, kind="ExternalInput")
    att = nc.dram_tensor("att", [dout, 2], bf16, kind="ExternalInput")
    out = nc.dram_tensor("out", [s, dout], f32, kind="ExternalOutput")

    const_pool = ctx.enter_context(tc.tile_pool(name="const", bufs=1))
    ph1_pool = ctx.enter_context(tc.tile_pool(name="ph1", bufs=1))
    dram_pool = ctx.enter_context(tc.tile_pool(name="dram", bufs=1, space="DRAM"))
    adj_pool = ctx.enter_context(tc.tile_pool(name="adj", bufs=1))
    q_pool = ctx.enter_context(tc.tile_pool(name="q", bufs=4))
    m_pool = ctx.enter_context(tc.tile_pool(name="m", bufs=3))
    adjr_pool = ctx.enter_context(tc.tile_pool(name="adjr", bufs=3))
    fin_pool = ctx.enter_context(tc.tile_pool(name="fin", bufs=2))

    # ---- Phase 1: input loads (xt first: it is on the collective's
    # critical path; adj tiles are emitted after the payload DMA) ----------
    w_sb = []
    x_sb = []
    for k in range(kc_n):
        xt_t = ph1_pool.tile([P, s], bf16, name="x_sb", tag=f"x_sb{k}")
        nc.sync.dma_start(xt_t[:], xt[k * P:(k + 1) * P, :])
        x_sb.append(xt_t)
    for k in range(kc_n):
        wt = ph1_pool.tile([P, dout], bf16, name="w_sb", tag=f"w_sb{k}")
        nc.sync.dma_start(wt[:], wmat[k * P:(k + 1) * P, :])
        w_sb.append(wt)
    att_sb = const_pool.tile([P, 2], bf16, name="att_sb")
    nc.sync.dma_start(att_sb[:], att[:])
    wt_sb = ph1_pool.tile([P, din], bf16, name="wt_sb")
    nc.sync.dma_start(wt_sb[:], wtr[:])

    warm_sb = const_pool.tile([1, 2], f32, name="warm_sb")
    nc.gpsimd.memset(warm_sb[:], 0.0)
    warm_dram = dram_pool.tile([1, 2], f32, name="warm_dram")
    nc.sync.dma_start(warm_dram[:], warm_sb[:])
    warmfull = dram_pool.tile([NCORES, 2], f32, addr_space="Shared",
                              name="warmfull")
    nc.gpsimd.collective_compute(
        "AllGather", ALU.bypass, replica_groups=[list(range(NCORES))],
        ins=[warm_dram.opt()], outs=[warmfull.opt()],
    )

    ident = const_pool.tile([P, P], f32, name="ident")
    masks.make_identity(nc, ident[:])
    identr_t = const_pool.tile([P, P], f32r, name="identr")
    nc.scalar.activation(identr_t[:], ident[:], AF.Copy)
    identr = identr_t[:]
    identb_t = const_pool.tile([2, 2], bf16, name="identb")
    nc.scalar.activation(identb_t[:], ident[:2, :2], AF.Copy)
    identb = identb_t[:]
    ones_f = const_pool.tile([1, P], f32, name="ones_f")
    nc.gpsimd.memset(ones_f[:], 1.0)
    ones_sb = const_pool.tile([1, P], f32r, name="ones_sb")
    nc.scalar.activation(ones_sb[:], ones_f[:], AF.Copy)

    # a_s/a_n via wsn = (attn.T @ W.T), then av = wsn_k.T @ x; h computed
    # directly in [node, d] orientation (no transposes needed)
    av_sb = ph1_pool.tile([2, s], f32r, name="av_sb")
    with tc.tile_pool(name="ph1ps", bufs=1, space="PSUM") as ph1_psum, \
         tc.tile_pool(name="ph1ps2", bufs=2, space="PSUM") as ph1_psum2:
        wsn_ps = ph1_psum.tile([2, din], f32, name="wsn_ps")
        nc.tensor.matmul(wsn_ps[:], att_sb[:], wt_sb[:],
                         start=True, stop=True)
        wsn_sb = ph1_pool.tile([2, din], bf16, name="wsn_sb")
        nc.scalar.activation(wsn_sb[:], wsn_ps[:], AF.Copy)
        wsnT_sb = []
        for k in range(kc_n):
            wT_ps = ph1_psum.tile([P, 2], bf16, name="wT_ps")
            nc.tensor.matmul(
                wT_ps[:], wsn_sb[:, k * P:(k + 1) * P], identb[:2, :2],
                is_transpose=True, start=True, stop=True,
            )
            wst = ph1_pool.tile([P, 2], bf16, name="wsnT_sb", tag=f"wsnT{k}")
            nc.scalar.activation(wst[:], wT_ps[:], AF.Copy)
            wsnT_sb.append(wst)
        for b in range(ib_n):
            av_ps = ph1_psum2.tile([2, nb], f32, name="av_ps")
            for k in range(kc_n):
                nc.tensor.matmul(
                    av_ps[:], wsnT_sb[k][:], x_sb[k][:, b * nb:(b + 1) * nb],
                    start=(k == 0), stop=(k == kc_n - 1),
                )
            nc.scalar.activation(av_sb[:, b * nb:(b + 1) * nb], av_ps[:], AF.Copy)

        # anT[p, c] = a_n of local node 128c + p;  expan = e^{a_n}
        anT_sb = ph1_pool.tile([P, sc_n], f32, name="anT_sb")
        for c in range(sc_n):
            avT_ps = ph1_psum.tile([P, 2], f32r, name="avT_ps")
            nc.tensor.matmul(
                avT_ps[:], av_sb[:, c * P:(c + 1) * P], identr[:2, :2],
                is_transpose=True, start=True, stop=True,
            )
            nc.scalar.activation(anT_sb[:, c:c + 1], avT_ps[:, 1:2], AF.Copy)
        expan_sb = ph1_pool.tile([P, sc_n], f32, name="expan_sb")
        nc.scalar.activation(expan_sb[:], anT_sb[:], AF.Exp, scale=1.0)

        # ---- Phase 2: combined gather payload [hw | ew | a_n | pad] -------
        hwan_sb = ph1_pool.tile([P, sc_n * PW], bf16, name="hwan_sb")
        nc.gpsimd.memset(hwan_sb[:], 0.0)
        for c in range(sc_n):
            hn_ps = ph1_psum2.tile([P, P], f32, name="hn_ps")
            for k in range(kc_n):
                nc.tensor.matmul(
                    hn_ps[:], x_sb[k][:, c * P:(c + 1) * P], w_sb[k][:],
                    start=(k == 0), stop=(k == kc_n - 1),
                )
            nc.scalar.activation(
                hwan_sb[:, c * PW:c * PW + dout], hn_ps[:], AF.Copy,
                scale=expan_sb[:, c:c + 1],
            )

        hwv = hwan_sb[:].rearrange("p (c d) -> p c d", d=PW)
        nc.scalar.activation(
            hwv[:, :, dout:dout + 1],
            expan_sb[:].rearrange("p c -> p c ()"), AF.Copy)
        nc.scalar.activation(
            hwv[:, :, dout + 1:dout + 2],
            anT_sb[:].rearrange("p c -> p c ()"), AF.Copy)

        hwan_dram = dram_pool.tile([s, PW], bf16, name="hwan_dram")
        nc.sync.dma_start(
            hwan_dram[:].rearrange("(p kl) d -> p (kl d)", kl=sc_n), hwan_sb[:])
        hwfull_dram = dram_pool.tile([n, PW], bf16, addr_space="Shared",
                                     name="hwfull")
        nc.gpsimd.collective_compute(
            "AllGather", ALU.bypass, replica_groups=[list(range(NCORES))],
            ins=[hwan_dram.opt()], outs=[hwfull_dram.opt()],
        )

        # ---- adjacency prefetch, all gated: g0/g1 on av_sb (t~15us, after
        # xt is consumed), the rest on c_sb (post-readback) so the payload
        # DMA, collective and readback never queue behind adj traffic ------
        adj_t = []
        for g in range(2):
            at = adj_pool.tile([P, GP * s], adt, name="adj_t", tag=f"adj{g}")
            nc.scalar.activation(at[0:1, 0:1], av_sb[0:1, 0:1], AF.Copy)
            nc.scalar.dma_start(
                at[:],
                adjt[g * GP * P:(g + 1) * GP * P, :].rearrange(
                    "(p q) i -> p (q i)", q=GP),
            )
            adj_t.append(at)

        # wb[p, i] = e^{0.8 a_s_i} broadcast to all partitions
        wrow_sb = ph1_pool.tile([1, s], f32r, name="wrow_sb")
        nc.scalar.activation(wrow_sb[:], av_sb[0:1, :], AF.Exp, scale=0.8)
        wb_sb = ph1_pool.tile([P, s], bf16, name="wb_sb")
        for b in range(ib_n):
            wb_ps = ph1_psum.tile([P, nb], f32, name="wb_ps")
            nc.tensor.matmul(
                wb_ps[:], ones_sb[:], wrow_sb[:, b * nb:(b + 1) * nb],
                start=True, stop=True,
            )
            nc.scalar.activation(wb_sb[:, b * nb:(b + 1) * nb], wb_ps[:], AF.Copy)

    # ---- Phase 3: readback of the gathered payload ------------------------
    # h2big[p, j*PW + d] = hwfull[node 128j + p, d]
    h2big = ph1_pool.tile([P, jc_n * PW], bf16, name="h2big")
    for cblk in range(NCORES):
        nc.sync.dma_start(
            h2big[:, cblk * sc_n * PW:(cblk + 1) * sc_n * PW],
            hwfull_dram[cblk * s:(cblk + 1) * s, :].rearrange(
                "(p kl) d -> p (kl d)", kl=sc_n),
        )
    h2v = h2big[:].rearrange("p (j d) -> p j d", d=PW)
    c_sb = ph1_pool.tile([P, jc_n], f32, name="c_sb")
    nc.scalar.activation(
        c_sb[:].rearrange("p j -> p j ()"), h2v[:, :, dout + 1:dout + 2],
        AF.Exp, scale=-0.8,
    )
    ew_sb = ph1_pool.tile([P, jc_n], bf16, name="ew_sb")
    nc.scalar.activation(
        ew_sb[:].rearrange("p j -> p j ()"), h2v[:, :, dout:dout + 1], AF.Copy)

    for g in range(2, g_n):
        at = adjr_pool.tile([P, GP * s], adt, name="adjr")
        nc.scalar.activation(at[0:1, 0:1], c_sb[0:1, 0:1], AF.Copy)
        nc.scalar.dma_start(
            at[:],
            adjt[g * GP * P:(g + 1) * GP * P, :].rearrange(
                "(p q) i -> p (q i)", q=GP),
        )
        adj_t.append(at)

    # ---- Phase 4: main loop over source-node chunks -----------------------
    if PLAN_B:
        with tc.tile_pool(name="acc", bufs=1, space="PSUM") as acc_psum:
            acc_ps = [acc_psum.tile([P, nb], f32, name=f"acc{sub}")
                      for sub in range(sc_n)]
            for j in range(jc_n):
                mt = m_pool.tile([P, s], bf16, name="mt")
                nc.vector.tensor_scalar_max(mt[:], wb_sb[:], c_sb[:, j:j + 1])
                qq = q_pool.tile([P, s], bf16, name="qq")
                eng = nc.gpsimd if (j % 2 == 1) else nc.vector
                eng.tensor_tensor(
                    qq[:], mt[:],
                    adj_t[j // GP][:, (j % GP) * s:(j % GP + 1) * s],
                    op=ALU.mult,
                )
                for sub in range(sc_n):
                    nc.tensor.matmul(
                        acc_ps[sub][:, 0:PW],
                        qq[:, sub * P:(sub + 1) * P],
                        h2big[:, j * PW:(j + 1) * PW],
                        start=(j == 0), stop=(j == jc_n - 1),
                    )
            for sub in range(sc_n):
                rec_sb = fin_pool.tile([P, 1], f32, name="rec_sb")
                nc.vector.reciprocal(rec_sb[:], acc_ps[sub][:, dout:dout + 1])
                oc_sb = fin_pool.tile([P, dout], f32, name="oc_sb")
                nc.scalar.activation(
                    oc_sb[:], acc_ps[sub][:, 0:dout], AF.Relu,
                    scale=rec_sb[:],
                )
                nc.sync.dma_start(out[sub * P:(sub + 1) * P, :], oc_sb[:])
        return

    # Plan A: weights = [hw] chunk, moving = q; separate row-sum matmuls.
    with tc.tile_pool(name="acc", bufs=1, space="PSUM") as acc_psum, \
         tc.tile_pool(name="tp", bufs=2, space="PSUM") as tp_psum:
        mm_ps = [acc_psum.tile([P, nb], f32, name=f"mm_ps{b}")
                 for b in range(ib_n)]
        rs_ps = [acc_psum.tile([1, nb], f32, name=f"rs_ps{b}")
                 for b in range(ib_n)]
        for j in range(jc_n):
            mt = m_pool.tile([P, s], bf16, name="mt")
            nc.vector.tensor_scalar_max(mt[:], wb_sb[:], c_sb[:, j:j + 1])
            qq = q_pool.tile([P, s], bf16, name="qq")
            nc.vector.tensor_tensor(
                qq[:], mt[:],
                adj_t[j // GP][:, (j % GP) * s:(j % GP + 1) * s],
                op=ALU.mult,
            )
            for b in range(ib_n):
                nc.tensor.matmul(
                    mm_ps[b][:], h2v[:, j, 0:dout], qq[:, b * nb:(b + 1) * nb],
                    start=(j == 0), stop=(j == jc_n - 1),
                )
            for b in range(ib_n):
                nc.tensor.matmul(
                    rs_ps[b][:], ew_sb[:, j:j + 1], qq[:, b * nb:(b + 1) * nb],
                    start=(j == 0), stop=(j == jc_n - 1),
                )

        # ---- Phase 5: normalize, relu, transpose out ----------------------
        rs_sb = ph1_pool.tile([1, s], f32, name="rs_sb")
        for b in range(ib_n):
            nc.scalar.activation(rs_sb[:, b * nb:(b + 1) * nb], rs_ps[b][:],
                                 AF.Copy)
        rsT_ps = tp_psum.tile([P, sc_n], f32, name="rsT_ps", tag="tp")
        for c in range(sc_n):
            nc.tensor.matmul(
                rsT_ps[:, c:c + 1], rs_sb[0:1, c * P:(c + 1) * P],
                ident[:1, :1], is_transpose=True, start=True, stop=True,
            )
        rrT_sb = ph1_pool.tile([P, sc_n], f32, name="rrT_sb")
        nc.vector.reciprocal(rrT_sb[:], rsT_ps[:])

        mo_sb = ph1_pool.tile([P, s], f32r, name="mo_sb")
        for b in range(ib_n):
            nc.scalar.activation(mo_sb[:, b * nb:(b + 1) * nb], mm_ps[b][:],
                                 AF.Copy)
        for c in range(sc_n):
            ot_ps = tp_psum.tile([P, P], f32r, name="ot_ps", tag="tp")
            nc.tensor.matmul(
                ot_ps[:], mo_sb[:, c * P:(c + 1) * P], identr[:],
                is_transpose=True, start=True, stop=True,
            )
            oc_sb = fin_pool.tile([P, dout], f32, name="oc_sb")
            nc.scalar.activation(oc_sb[:], ot_ps[:], AF.Relu,
                                 scale=rrT_sb[:, c:c + 1])
            nc.sync.dma_start(out[c * P:(c + 1) * P, :], oc_sb[:])


def build_nc(n=N, s=S, din=DIN, dout=DOUT):
    from contextlib import ExitStack

    import concourse.bacc as bacc
    import concourse.tile as tile

    nc = bacc.Bacc(
        "TRN2",
        target_bir_lowering=False,
        debug=False,
        num_devices=NCORES,
    )
    with tile.TileContext(nc) as tc, ExitStack() as ctx:
        _emit(nc, tc, ctx, n, s, din, dout)
    nc.compile()
    return nc


def prep_adjt(adj_slab):
    """[s, n] adj row-slab -> [n, s] with GP-chunk interleave.

    DRAM row g*1024 + 8p + q holds adjT row (8g+q)*128 + p, so one DMA per
    super-chunk g reads 8 consecutive rows per partition (16 KB descriptors).
    """
    import ml_dtypes

    dt = ml_dtypes.float8_e4m3 if ADJ_FP8 else ml_dtypes.bfloat16
    adjt = adj_slab.T  # [n, s]
    n, ss = adjt.shape
    P = 128
    adjt = adjt.reshape(n // (GP * P), GP, P, ss).transpose(0, 2, 1, 3)
    return np.ascontiguousarray(adjt.reshape(n, ss).astype(dt))


def make_in_maps(x, adj, W, attn_self, attn_neigh, s=S):
    import ml_dtypes

    att = np.concatenate([attn_self, attn_neigh], axis=1).astype(ml_dtypes.bfloat16)
    wmat = np.ascontiguousarray(W.astype(ml_dtypes.bfloat16))
    wtm = np.ascontiguousarray(W.T.astype(ml_dtypes.bfloat16))
    in_maps = []
    for c in range(NCORES):
        sl = slice(c * s, (c + 1) * s)
        in_maps.append({
            "adjt": prep_adjt(adj[sl, :]),
            "xt": np.ascontiguousarray(x[sl, :].T.astype(ml_dtypes.bfloat16)),
            "wmat": wmat,
            "wt": wtm,
            "att": att,
        })
    return in_maps


def kernel(x, adj, W, attn_self, attn_neigh):
    from concourse.bass_utils import run_bass_kernel_spmd

    x = np.asarray(x, dtype=np.float32)
    adj = np.asarray(adj, dtype=np.float32)
    W = np.asarray(W, dtype=np.float32)
    attn_self = np.asarray(attn_self, dtype=np.float32)
    attn_neigh = np.asarray(attn_neigh, dtype=np.float32)

    nc = build_nc()
    in_maps = make_in_maps(x, adj, W, attn_self, attn_neigh)
    res = run_bass_kernel_spmd(nc, in_maps, list(range(NCORES)))
    return np.concatenate([res.results[c]["out"] for c in range(NCORES)], axis=0)


# revision 20
# speedup vs baseline: 1.1025x; 1.1025x over previous
"""AttentiveGraphConvolution (GAT-style layer) on 8 trn2 NeuronCores.

Math (reference):
    h   = x @ W                       [N, D]
    a_s = h @ attn_self               [N, 1]
    a_n = h @ attn_neigh              [N, 1]
    e   = leaky_relu(a_s + a_n.T, 0.2)
    e   = e + NEG_INF * (1 - adj)
    out = relu(softmax(e, -1) @ h)

Reformulation (exact up to rounding): with exp(leaky(s)) = max(e^s, e^{0.2 s}),

    exp(e_ij) = e^{0.2 a_s_i} * e^{a_n_j} * max(e^{0.8 a_s_i}, e^{-0.8 a_n_j})

The row factor e^{0.2 a_s_i} cancels in the softmax.  With
    wb_i = e^{0.8 a_s_i}      (per output node, broadcast tile)
    c_j  = e^{-0.8 a_n_j}     (per source node, per-partition scalar)
    ew_j = e^{a_n_j}          (row-sum weights)
    hw_j = ew_j * h_j         (pre-scaled h rows)
one DVE op per adjacency chunk produces q_ji = max(wb_i, c_j) * adjT_ji and

    out_i = relu( (sum_j hw_j q_ji) / (sum_j ew_j q_ji) )

Sharding: output rows i across 8 cores; each core streams its adjT slab
[n, s] (bf16, binary-exact).  A single AllGather moves the combined
[hw | ew | a_n] payload (~270 KB/core bf16); everything else is local.
"""

import numpy as np

N = 8192
DIN = 512
DOUT = 128
NCORES = 8
S = N // NCORES     # 1024 output rows per core
GP = 8              # adjacency chunks per super-chunk (one DMA each)
PW = 132            # payload row width: 128 hw | ew | a_n | 2 pad

PLAN_B = True      # True: q-as-weights matmuls (num+den in one pass)
ADJ_FP8 = False     # True: stream adjacency as fp8e4 (binary-exact)


def _emit(nc, tc, ctx, n, s, din, dout):
    from concourse import masks, mybir

    f32 = mybir.dt.float32
    f32r = mybir.dt.float32r
    bf16 = mybir.dt.bfloat16
    fp8 = mybir.dt.float8e4
    adt = fp8 if ADJ_FP8 else bf16
    AF = mybir.ActivationFunctionType
    ALU = mybir.AluOpType

    P = 128
    jc_n = n // P           # 64 source-node chunks
    sc_n = s // P           # 8 chunks in the local row slab
    kc_n = din // P         # 4 contraction chunks for x @ W
    nb = 512                # matmul moving-dim block
    ib_n = s // nb          # 2 i blocks per core
    g_n = jc_n // GP        # 8 adj super-chunks

    adjt = nc.dram_tensor("adjt", [n, s], adt, kind="ExternalInput")
    xt = nc.dram_tensor("xt", [din, s], bf16, kind="ExternalInput")
    wmat = nc.dram_tensor("wmat", [din, dout], bf16, kind="ExternalInput")
    wtr = nc.dram_tensor("wt", [dout, din], bf16, kind="ExternalInput")
    att = nc.dram_tensor("att", [dout, 2], bf16, kind="ExternalInput")
    out = nc.dram_tensor("out", [s, dout], f32, kind="ExternalOutput")

    const_pool = ctx.enter_context(tc.tile_pool(name="const", bufs=1))
    ph1_pool = ctx.enter_context(tc.tile_pool(name="ph1", bufs=1))
    dram_pool = ctx.enter_context(tc.tile_pool(name="dram", bufs=1, space="DRAM"))
    adj_pool = ctx.enter_context(tc.tile_pool(name="adj", bufs=1))
    q_pool = ctx.enter_context(tc.tile_pool(name="q", bufs=4))
    m_pool = ctx.enter_context(tc.tile_pool(name="m", bufs=3))
    adjr_pool = ctx.enter_context(tc.tile_pool(name="adjr", bufs=3))
    fin_pool = ctx.enter_context(tc.tile_pool(name="fin", bufs=2))

    # ---- Phase 1: input loads (xt first: it is on the collective's
    # critical path; adj tiles are emitted after the payload DMA) ----------
    w_sb = []
    x_sb = []
    for k in range(kc_n):
        xt_t = ph1_pool.tile([P, s], bf16, name="x_sb", tag=f"x_sb{k}")
        nc.sync.dma_start(xt_t[:], xt[k * P:(k + 1) * P, :])
        x_sb.append(xt_t)
    for k in range(kc_n):
        wt = ph1_pool.tile([P, dout], bf16, name="w_sb", tag=f"w_sb{k}")
        nc.sync.dma_start(wt[:], wmat[k * P:(k + 1) * P, :])
        w_sb.append(wt)
    att_sb = const_pool.tile([P, 2], bf16, name="att_sb")
    nc.sync.dma_start(att_sb[:], att[:])
    wt_sb = ph1_pool.tile([P, din], bf16, name="wt_sb")
    nc.sync.dma_start(wt_sb[:], wtr[:])

    warm_sb = const_pool.tile([1, 2], f32, name="warm_sb")
    nc.gpsimd.memset(warm_sb[:], 0.0)
    warm_dram = dram_pool.tile([1, 2], f32, name="warm_dram")
    nc.sync.dma_start(warm_dram[:], warm_sb[:])
    warmfull = dram_pool.tile([NCORES, 2], f32, addr_space="Shared",
                              name="warmfull")
    nc.gpsimd.collective_compute(
        "AllGather", ALU.bypass, replica_groups=[list(range(NCORES))],
        ins=[warm_dram.opt()], outs=[warmfull.opt()],
    )

    ident = const_pool.tile([P, P], f32, name="ident")
    masks.make_identity(nc, ident[:])
    identr_t = const_pool.tile([P, P], f32r, name="identr")
    nc.scalar.activation(identr_t[:], ident[:], AF.Copy)
    identr = identr_t[:]
    identb_t = const_pool.tile([2, 2], bf16, name="identb")
    nc.scalar.activation(identb_t[:], ident[:2, :2], AF.Copy)
    identb = identb_t[:]
    ones_f = const_pool.tile([1, P], f32, name="ones_f")
    nc.gpsimd.memset(ones_f[:], 1.0)
    ones_sb = const_pool.tile([1, P], f32r, name="ones_sb")
    nc.scalar.activation(ones_sb[:], ones_f[:], AF.Copy)

    # a_s/a_n via wsn = (attn.T @ W.T), then av = wsn_k.T @ x; h computed
    # directly in [node, d] orientation (no transposes needed)
    av_sb = ph1_pool.tile([2, s], f32r, name="av_sb")
    with tc.tile_pool(name="ph1ps", bufs=1, space="PSUM") as ph1_psum, \
         tc.tile_pool(name="ph1ps2", bufs=2, space="PSUM") as ph1_psum2:
        wsn_ps = ph1_psum.tile([2, din], f32, name="wsn_ps")
        nc.tensor.matmul(wsn_ps[:], att_sb[:], wt_sb[:],
                         start=True, stop=True)
        wsn_sb = ph1_pool.tile([2, din], bf16, name="wsn_sb")
        nc.scalar.activation(wsn_sb[:], wsn_ps[:], AF.Copy)
        wsnT_sb = []
        for k in range(kc_n):
            wT_ps = ph1_psum.tile([P, 2], bf16, name="wT_ps")
            nc.tensor.matmul(
                wT_ps[:], wsn_sb[:, k * P:(k + 1) * P], identb[:2, :2],
                is_transpose=True, start=True, stop=True,
            )
            wst = ph1_pool.tile([P, 2], bf16, name="wsnT_sb", tag=f"wsnT{k}")
            nc.scalar.activation(wst[:], wT_ps[:], AF.Copy)
            wsnT_sb.append(wst)
        for b in range(ib_n):
            av_ps = ph1_psum2.tile([2, nb], f32, name="av_ps")
            for k in range(kc_n):
                nc.tensor.matmul(
                    av_ps[:], wsnT_sb[k][:], x_sb[k][:, b * nb:(b + 1) * nb],
                    start=(k == 0), stop=(k == kc_n - 1),
                )
            nc.scalar.activation(av_sb[:, b * nb:(b + 1) * nb], av_ps[:], AF.Copy)

        # anT[p, c] = a_n of local node 128c + p;  expan = e^{a_n}
        anT_sb = ph1_pool.tile([P, sc_n], f32, name="anT_sb")
        for c in range(sc_n):
            avT_ps = ph1_psum.tile([P, 2], f32r, name="avT_ps")
            nc.tensor.matmul(
                avT_ps[:], av_sb[:, c * P:(c + 1) * P], identr[:2, :2],
                is_transpose=True, start=True, stop=True,
            )
            nc.scalar.activation(anT_sb[:, c:c + 1], avT_ps[:, 1:2], AF.Copy)
        expan_sb = ph1_pool.tile([P, sc_n], f32, name="expan_sb")
        nc.scalar.activation(expan_sb[:], anT_sb[:], AF.Exp, scale=1.0)

        # ---- Phase 2: combined gather payload [hw | ew | a_n | pad] -------
        hwan_sb = ph1_pool.tile([P, sc_n * PW], bf16, name="hwan_sb")
        nc.gpsimd.memset(hwan_sb[:], 0.0)
        for c in range(sc_n):
            hn_ps = ph1_psum2.tile([P, P], f32, name="hn_ps")
            for k in range(kc_n):
                nc.tensor.matmul(
                    hn_ps[:], x_sb[k][:, c * P:(c + 1) * P], w_sb[k][:],
                    start=(k == 0), stop=(k == kc_n - 1),
                )
            nc.scalar.activation(
                hwan_sb[:, c * PW:c * PW + dout], hn_ps[:], AF.Copy,
                scale=expan_sb[:, c:c + 1],
            )

        hwv = hwan_sb[:].rearrange("p (c d) -> p c d", d=PW)
        nc.scalar.activation(
            hwv[:, :, dout:dout + 1],
            expan_sb[:].rearrange("p c -> p c ()"), AF.Copy)
        nc.scalar.activation(
            hwv[:, :, dout + 1:dout + 2],
            anT_sb[:].rearrange("p c -> p c ()"), AF.Copy)

        hwan_dram = dram_pool.tile([s, PW], bf16, name="hwan_dram")
        nc.sync.dma_start(
            hwan_dram[:].rearrange("(p kl) d -> p (kl d)", kl=sc_n), hwan_sb[:])
        hwfull_dram = dram_pool.tile([n, PW], bf16, addr_space="Shared",
                                     name="hwfull")
        nc.gpsimd.collective_compute(
            "AllGather", ALU.bypass, replica_groups=[list(range(NCORES))],
            ins=[hwan_dram.opt()], outs=[hwfull_dram.opt()],
        )

        # ---- adjacency prefetch, all gated: g0/g1 on av_sb (t~15us, after
        # xt is consumed), the rest on c_sb (post-readback) so the payload
        # DMA, collective and readback never queue behind adj traffic ------
        adj_t = []
        for g in range(2):
            at = adj_pool.tile([P, GP * s], adt, name="adj_t", tag=f"adj{g}")
            nc.scalar.activation(at[0:1, 0:1], av_sb[0:1, 0:1], AF.Copy)
            nc.scalar.dma_start(
                at[:],
                adjt[g * GP * P:(g + 1) * GP * P, :].rearrange(
                    "(p q) i -> p (q i)", q=GP),
            )
            adj_t.append(at)

        # wb[p, i] = e^{0.8 a_s_i} broadcast to all partitions
        wrow_sb = ph1_pool.tile([1, s], f32r, name="wrow_sb")
        nc.scalar.activation(wrow_sb[:], av_sb[0:1, :], AF.Exp, scale=0.8)
        wb_sb = ph1_pool.tile([P, s], bf16, name="wb_sb")
        for b in range(ib_n):
            wb_ps = ph1_psum.tile([P, nb], f32, name="wb_ps")
            nc.tensor.matmul(
                wb_ps[:], ones_sb[:], wrow_sb[:, b * nb:(b + 1) * nb],
                start=True, stop=True,
            )
            nc.scalar.activation(wb_sb[:, b * nb:(b + 1) * nb], wb_ps[:], AF.Copy)

    # ---- Phase 3: readback of the gathered payload ------------------------
    # h2big[p, j*PW + d] = hwfull[node 128j + p, d]
    h2big = ph1_pool.tile([P, jc_n * PW], bf16, name="h2big")
    for cblk in range(NCORES):
        nc.sync.dma_start(
            h2big[:, cblk * sc_n * PW:(cblk + 1) * sc_n * PW],
            hwfull_dram[cblk * s:(cblk + 1) * s, :].rearrange(
                "(p kl) d -> p (kl d)", kl=sc_n),
        )
    h2v = h2big[:].rearrange("p (j d) -> p j d", d=PW)
    c_sb = ph1_pool.tile([P, jc_n], f32, name="c_sb")
    nc.scalar.activation(
        c_sb[:].rearrange("p j -> p j ()"), h2v[:, :, dout + 1:dout + 2],
        AF.Exp, scale=-0.8,
    )
    ew_sb = ph1_pool.tile([P, jc_n], bf16, name="ew_sb")
    nc.scalar.activation(
        ew_sb[:].rearrange("p j -> p j ()"), h2v[:, :, dout:dout + 1], AF.Copy)

    for g in range(2, g_n):
        at = adjr_pool.tile([P, GP * s], adt, name="adjr")
        nc.scalar.activation(at[0:1, 0:1], c_sb[0:1, 0:1], AF.Copy)
        nc.scalar.dma_start(
            at[:],
            adjt[g * GP * P:(g + 1) * GP * P, :].rearrange(
                "(p q) i -> p (q i)", q=GP),
        )
        adj_t.append(at)

    # ---- Phase 4: main loop over source-node chunks -----------------------
    if PLAN_B:
        with tc.tile_pool(name="acc", bufs=1, space="PSUM") as acc_psum:
            acc_ps = [acc_psum.tile([P, nb], f32, name=f"acc{sub}")
                      for sub in range(sc_n)]
            for j in range(jc_n):
                mt = m_pool.tile([P, s], bf16, name="mt")
                nc.vector.tensor_scalar_max(mt[:], wb_sb[:], c_sb[:, j:j + 1])
                qq = q_pool.tile([P, s], bf16, name="qq")
                nc.vector.tensor_tensor(
                    qq[:], mt[:],
                    adj_t[j // GP][:, (j % GP) * s:(j % GP + 1) * s],
                    op=ALU.mult,
                )
                for sub in range(sc_n):
                    nc.tensor.matmul(
                        acc_ps[sub][:, 0:PW],
                        qq[:, sub * P:(sub + 1) * P],
                        h2big[:, j * PW:(j + 1) * PW],
                        start=(j == 0), stop=(j == jc_n - 1),
                    )
            for sub in range(sc_n):
                rec_sb = fin_pool.tile([P, 1], f32, name="rec_sb")
                nc.vector.reciprocal(rec_sb[:], acc_ps[sub][:, dout:dout + 1])
                oc_sb = fin_pool.tile([P, dout], f32, name="oc_sb")
                nc.scalar.activation(
                    oc_sb[:], acc_ps[sub][:, 0:dout], AF.Relu,
                    scale=rec_sb[:],
                )
                nc.sync.dma_start(out[sub * P:(sub + 1) * P, :], oc_sb[:])
        return

    # Plan A: weights = [hw] chunk, moving = q; separate row-sum matmuls.
    with tc.tile_pool(name="acc", bufs=1, space="PSUM") as acc_psum, \
         tc.tile_pool(name="tp", bufs=2, space="PSUM") as tp_psum:
        mm_ps = [acc_psum.tile([P, nb], f32, name=f"mm_ps{b}")
                 for b in range(ib_n)]
        rs_ps = [acc_psum.tile([1, nb], f32, name=f"rs_ps{b}")
                 for b in range(ib_n)]
        for j in range(jc_n):
            mt = m_pool.tile([P, s], bf16, name="mt")
            nc.vector.tensor_scalar_max(mt[:], wb_sb[:], c_sb[:, j:j + 1])
            qq = q_pool.tile([P, s], bf16, name="qq")
            nc.vector.tensor_tensor(
                qq[:], mt[:],
                adj_t[j // GP][:, (j % GP) * s:(j % GP + 1) * s],
                op=ALU.mult,
            )
            for b in range(ib_n):
                nc.tensor.matmul(
                    mm_ps[b][:], h2v[:, j, 0:dout], qq[:, b * nb:(b + 1) * nb],
                    start=(j == 0), stop=(j == jc_n - 1),
                )
            for b in range(ib_n):
                nc.tensor.matmul(
                    rs_ps[b][:], ew_sb[:, j:j + 1], qq[:, b * nb:(b + 1) * nb],
                    start=(j == 0), stop=(j == jc_n - 1),
                )

        # ---- Phase 5: normalize, relu, transpose out ----------------------
        rs_sb = ph1_pool.tile([1, s], f32, name="rs_sb")
        for b in range(ib_n):
            nc.scalar.activation(rs_sb[:, b * nb:(b + 1) * nb], rs_ps[b][:],
                                 AF.Copy)
        rsT_ps = tp_psum.tile([P, sc_n], f32, name="rsT_ps", tag="tp")
        for c in range(sc_n):
            nc.tensor.matmul(
                rsT_ps[:, c:c + 1], rs_sb[0:1, c * P:(c + 1) * P],
                ident[:1, :1], is_transpose=True, start=True, stop=True,
            )
        rrT_sb = ph1_pool.tile([P, sc_n], f32, name="rrT_sb")
        nc.vector.reciprocal(rrT_sb[:], rsT_ps[:])

        mo_sb = ph1_pool.tile([P, s], f32r, name="mo_sb")
        for b in range(ib_n):
            nc.scalar.activation(mo_sb[:, b * nb:(b + 1) * nb], mm_ps[b][:],
                                 AF.Copy)
        for c in range(sc_n):
            ot_ps = tp_psum.tile([P, P], f32r, name="ot_ps", tag="tp")
            nc.tensor.matmul(
                ot_ps[:], mo_sb[:, c * P:(c + 1) * P], identr[:],
                is_transpose=True, start=True, stop=True,
            )
            oc_sb = fin_pool.tile([P, dout], f32, name="oc_sb")
            nc.scalar.activation(oc_sb[:], ot_ps[:], AF.Relu,
                                 scale=rrT_sb[:, c:c + 1])
            nc.sync.dma_start(out[c * P:(c + 1) * P, :], oc_sb[:])


def build_nc(n=N, s=S, din=DIN, dout=DOUT):
    from contextlib import ExitStack

    import concourse.bacc as bacc
    import concourse.tile as tile

    nc = bacc.Bacc(
        "TRN2",
        target_bir_lowering=False,
        debug=False,
        num_devices=NCORES,
    )
    with tile.TileContext(nc) as tc, ExitStack() as ctx:
        _emit(nc, tc, ctx, n, s, din, dout)
    nc.compile()
    return nc


def prep_adjt(adj_slab):
    """[s, n] adj row-slab -> [n, s] with GP-chunk interleave.

    DRAM row g*1024 + 8p + q holds adjT row (8g+q)*128 + p, so one DMA per
    super-chunk g reads 8 consecutive rows per partition (16 KB descriptors).
    """
    import ml_dtypes

    dt = ml_dtypes.float8_e4m3 if ADJ_FP8 else ml_dtypes.bfloat16
    adjt = adj_slab.T  # [n, s]
    n, ss = adjt.shape
    P = 128
    adjt = adjt.reshape(n // (GP * P), GP, P, ss).transpose(0, 2, 1, 3)
    return np.ascontiguousarray(adjt.reshape(n, ss).astype(dt))


def make_in_maps(x, adj, W, attn_self, attn_neigh, s=S):
    import ml_dtypes

    att = np.concatenate([attn_self, attn_neigh], axis=1).astype(ml_dtypes.bfloat16)
    wmat = np.ascontiguousarray(W.astype(ml_dtypes.bfloat16))
    wtm = np.ascontiguousarray(W.T.astype(ml_dtypes.bfloat16))
    in_maps = []
    for c in range(NCORES):
        sl = slice(c * s, (c + 1) * s)
        in_maps.append({
            "adjt": prep_adjt(adj[sl, :]),
            "xt": np.ascontiguousarray(x[sl, :].T.astype(ml_dtypes.bfloat16)),
            "wmat": wmat,
            "wt": wtm,
            "att": att,
        })
    return in_maps


def kernel(x, adj, W, attn_self, attn_neigh):
    from concourse.bass_utils import run_bass_kernel_spmd

    x = np.asarray(x, dtype=np.float32)
    adj = np.asarray(adj, dtype=np.float32)
    W = np.asarray(W, dtype=np.float32)
    attn_self = np.asarray(attn_self, dtype=np.float32)
    attn_neigh = np.asarray(attn_neigh, dtype=np.float32)

    nc = build_nc()
    in_maps = make_in_maps(x, adj, W, attn_self, attn_neigh)
    res = run_bass_kernel_spmd(nc, in_maps, list(range(NCORES)))
    return np.concatenate([res.results[c]["out"] for c in range(NCORES)], axis=0)


# revision 22
# speedup vs baseline: 1.1704x; 1.0616x over previous
"""AttentiveGraphConvolution (GAT-style layer) on 8 trn2 NeuronCores.

Math (reference):
    h   = x @ W                       [N, D]
    a_s = h @ attn_self               [N, 1]
    a_n = h @ attn_neigh              [N, 1]
    e   = leaky_relu(a_s + a_n.T, 0.2)
    e   = e + NEG_INF * (1 - adj)
    out = relu(softmax(e, -1) @ h)

Reformulation (exact up to rounding): with exp(leaky(s)) = max(e^s, e^{0.2 s}),

    exp(e_ij) = e^{0.2 a_s_i} * e^{a_n_j} * max(e^{0.8 a_s_i}, e^{-0.8 a_n_j})

The row factor e^{0.2 a_s_i} cancels in the softmax.  With
    wb_i = e^{0.8 a_s_i}      (per output node, broadcast tile)
    c_j  = e^{-0.8 a_n_j}     (per source node, per-partition scalar)
    ew_j = e^{a_n_j}          (row-sum weights)
    hw_j = ew_j * h_j         (pre-scaled h rows)
one DVE op per adjacency chunk produces q_ji = max(wb_i, c_j) * adjT_ji and

    out_i = relu( (sum_j hw_j q_ji) / (sum_j ew_j q_ji) )

Sharding: output rows i across 8 cores; each core streams its adjT slab
[n, s] (bf16, binary-exact).  A single AllGather moves the combined
[hw | ew | a_n] payload (~270 KB/core bf16); everything else is local.
"""

import numpy as np

N = 8192
DIN = 512
DOUT = 128
NCORES = 8
S = N // NCORES     # 1024 output rows per core
GP = 8              # adjacency chunks per super-chunk (one DMA each)
PW = 132            # payload row width: 128 hw | ew | a_n | 2 pad

PLAN_B = True      # True: q-as-weights matmuls (num+den in one pass)
ADJ_FP8 = False     # True: stream adjacency as fp8e4 (binary-exact)


def _emit(nc, tc, ctx, n, s, din, dout):
    from concourse import masks, mybir

    f32 = mybir.dt.float32
    f32r = mybir.dt.float32r
    bf16 = mybir.dt.bfloat16
    fp8 = mybir.dt.float8e4
    adt = fp8 if ADJ_FP8 else bf16
    AF = mybir.ActivationFunctionType
    ALU = mybir.AluOpType

    P = 128
    jc_n = n // P           # 64 source-node chunks
    sc_n = s // P           # 8 chunks in the local row slab
    kc_n = din // P         # 4 contraction chunks for x @ W
    nb = 512                # matmul moving-dim block
    ib_n = s // nb          # 2 i blocks per core
    g_n = jc_n // GP        # 8 adj super-chunks

    adjt = nc.dram_tensor("adjt", [n, s], adt, kind="ExternalInput")
    xt = nc.dram_tensor("xt", [din, s], bf16, kind="ExternalInput")
    wmat = nc.dram_tensor("wmat", [din, dout], bf16, kind="ExternalInput")
    wtr = nc.dram_tensor("wt", [dout, din], bf16, kind="ExternalInput")
    att = nc.dram_tensor("att", [dout, 2], bf16, kind="ExternalInput")
    out = nc.dram_tensor("out", [s, dout], f32, kind="ExternalOutput")

    const_pool = ctx.enter_context(tc.tile_pool(name="const", bufs=1))
    ph1_pool = ctx.enter_context(tc.tile_pool(name="ph1", bufs=1))
    dram_pool = ctx.enter_context(tc.tile_pool(name="dram", bufs=1, space="DRAM"))
    adj_pool = ctx.enter_context(tc.tile_pool(name="adj", bufs=1))
    q_pool = ctx.enter_context(tc.tile_pool(name="q", bufs=4))
    m_pool = ctx.enter_context(tc.tile_pool(name="m", bufs=3))
    adjr_pool = ctx.enter_context(tc.tile_pool(name="adjr", bufs=3))
    fin_pool = ctx.enter_context(tc.tile_pool(name="fin", bufs=2))

    # ---- Phase 1: input loads (xt first: it is on the collective's
    # critical path; adj tiles are emitted after the payload DMA) ----------
    w_sb = []
    x_sb = []
    for k in range(kc_n):
        xt_t = ph1_pool.tile([P, s], bf16, name="x_sb", tag=f"x_sb{k}")
        nc.sync.dma_start(xt_t[:], xt[k * P:(k + 1) * P, :])
        x_sb.append(xt_t)
    for k in range(kc_n):
        wt = ph1_pool.tile([P, dout], bf16, name="w_sb", tag=f"w_sb{k}")
        nc.sync.dma_start(wt[:], wmat[k * P:(k + 1) * P, :])
        w_sb.append(wt)
    att_sb = const_pool.tile([P, 2], bf16, name="att_sb")
    nc.sync.dma_start(att_sb[:], att[:])
    wt_sb = ph1_pool.tile([P, din], bf16, name="wt_sb")
    nc.sync.dma_start(wt_sb[:], wtr[:])

    warm_sb = const_pool.tile([1, 2], f32, name="warm_sb")
    nc.gpsimd.memset(warm_sb[:], 0.0)
    warm_dram = dram_pool.tile([1, 2], f32, name="warm_dram")
    nc.sync.dma_start(warm_dram[:], warm_sb[:])
    warmfull = dram_pool.tile([NCORES, 2], f32, addr_space="Shared",
                              name="warmfull")
    nc.gpsimd.collective_compute(
        "AllGather", ALU.bypass, replica_groups=[list(range(NCORES))],
        ins=[warm_dram.opt()], outs=[warmfull.opt()],
    )

    ident = const_pool.tile([P, P], f32, name="ident")
    masks.make_identity(nc, ident[:])
    identr_t = const_pool.tile([P, P], f32r, name="identr")
    nc.scalar.activation(identr_t[:], ident[:], AF.Copy)
    identr = identr_t[:]
    identb_t = const_pool.tile([2, 2], bf16, name="identb")
    nc.scalar.activation(identb_t[:], ident[:2, :2], AF.Copy)
    identb = identb_t[:]
    ones_f = const_pool.tile([1, P], f32, name="ones_f")
    nc.gpsimd.memset(ones_f[:], 1.0)
    ones_sb = const_pool.tile([1, P], f32r, name="ones_sb")
    nc.scalar.activation(ones_sb[:], ones_f[:], AF.Copy)

    # wsn = (attn.T @ W.T) -> per-k [W | ws | wn] moving tiles so the h
    # matmul also yields a_s/a_n per node (cols 128/129) with no transposes
    with tc.tile_pool(name="ph1ps", bufs=1, space="PSUM") as ph1_psum, \
         tc.tile_pool(name="ph1ps2", bufs=2, space="PSUM") as ph1_psum2:
        wsn_ps = ph1_psum.tile([2, din], f32, name="wsn_ps")
        nc.tensor.matmul(wsn_ps[:], att_sb[:], wt_sb[:],
                         start=True, stop=True)
        wsn_sb = ph1_pool.tile([2, din], bf16, name="wsn_sb")
        nc.scalar.activation(wsn_sb[:], wsn_ps[:], AF.Copy)
        wext = []
        for k in range(kc_n):
            wT_ps = ph1_psum.tile([P, 2], bf16, name="wT_ps")
            nc.tensor.matmul(
                wT_ps[:], wsn_sb[:, k * P:(k + 1) * P], identb[:2, :2],
                is_transpose=True, start=True, stop=True,
            )
            wx = ph1_pool.tile([P, dout + 2], bf16, name="wext", tag=f"wext{k}")
            nc.scalar.activation(wx[:, 0:dout], w_sb[k][:], AF.Copy)
            nc.scalar.activation(wx[:, dout:dout + 2], wT_ps[:], AF.Copy)
            wext.append(wx)

        as_sb = ph1_pool.tile([P, sc_n], f32, name="as_sb")

        # ---- Phase 2: combined gather payload [hw | ew | a_n | pad] -------
        hwan_sb = ph1_pool.tile([P, sc_n * PW], bf16, name="hwan_sb")
        nc.gpsimd.memset(hwan_sb[:], 0.0)
        for c in range(sc_n):
            hn_ps = ph1_psum2.tile([P, dout + 2], f32, name="hn_ps")
            for k in range(kc_n):
                nc.tensor.matmul(
                    hn_ps[:], x_sb[k][:, c * P:(c + 1) * P], wext[k][:],
                    start=(k == 0), stop=(k == kc_n - 1),
                )
            expc = ph1_pool.tile([P, 1], f32, name="expc")
            nc.scalar.activation(expc[:], hn_ps[:, dout + 1:dout + 2],
                                 AF.Exp, scale=1.0)
            nc.scalar.activation(
                hwan_sb[:, c * PW:c * PW + dout], hn_ps[:, 0:dout], AF.Copy,
                scale=expc[:],
            )
            nc.scalar.activation(
                hwan_sb[:, c * PW + dout:c * PW + dout + 1], expc[:], AF.Copy)
            nc.scalar.activation(
                hwan_sb[:, c * PW + dout + 1:c * PW + dout + 2],
                hn_ps[:, dout + 1:dout + 2], AF.Copy)
            nc.scalar.activation(as_sb[:, c:c + 1], hn_ps[:, dout:dout + 1],
                                 AF.Copy)

        hwan_dram = dram_pool.tile([s, PW], bf16, name="hwan_dram")
        nc.sync.dma_start(
            hwan_dram[:].rearrange("(p kl) d -> p (kl d)", kl=sc_n), hwan_sb[:])
        hwfull_dram = dram_pool.tile([n, PW], bf16, addr_space="Shared",
                                     name="hwfull")
        nc.gpsimd.collective_compute(
            "AllGather", ALU.bypass, replica_groups=[list(range(NCORES))],
            ins=[hwan_dram.opt()], outs=[hwfull_dram.opt()],
        )

        # ---- adjacency prefetch, all gated: g0/g1 on av_sb (t~15us, after
        # xt is consumed), the rest on c_sb (post-readback) so the payload
        # DMA, collective and readback never queue behind adj traffic ------
        adj_t = []
        for g in range(2):
            at = adj_pool.tile([P, GP * s], adt, name="adj_t", tag=f"adj{g}")
            nc.scalar.activation(at[0:1, 0:1], as_sb[0:1, 0:1], AF.Copy)
            nc.scalar.dma_start(
                at[:],
                adjt[g * GP * P:(g + 1) * GP * P, :].rearrange(
                    "(p q) i -> p (q i)", q=GP),
            )
            adj_t.append(at)


    # ---- Phase 3: readback of the gathered payload ------------------------
    # h2big[p, j*PW + d] = hwfull[node 128j + p, d]
    h2big = ph1_pool.tile([P, jc_n * PW], bf16, name="h2big")
    for cblk in range(NCORES):
        nc.sync.dma_start(
            h2big[:, cblk * sc_n * PW:(cblk + 1) * sc_n * PW],
            hwfull_dram[cblk * s:(cblk + 1) * s, :].rearrange(
                "(p kl) d -> p (kl d)", kl=sc_n),
        )
    h2v = h2big[:].rearrange("p (j d) -> p j d", d=PW)
    c_sb = ph1_pool.tile([P, jc_n], f32, name="c_sb")
    nc.scalar.activation(
        c_sb[:].rearrange("p j -> p j ()"), h2v[:, :, dout + 1:dout + 2],
        AF.Exp, scale=-0.8,
    )
    ew_sb = ph1_pool.tile([P, jc_n], bf16, name="ew_sb")
    nc.scalar.activation(
        ew_sb[:].rearrange("p j -> p j ()"), h2v[:, :, dout:dout + 1], AF.Copy)

    # wb[p, i] = e^{0.8 a_s_i}: transpose a_s per-node cols into a row,
    # exp, then broadcast across partitions (runs in the gather window)
    with tc.tile_pool(name="wbps", bufs=2, space="PSUM") as wb_psum:
        wrow_ps = wb_psum.tile([1, s], f32, name="wrow_ps")
        for c in range(sc_n):
            nc.tensor.matmul(
                wrow_ps[0:1, c * P:(c + 1) * P], as_sb[:, c:c + 1],
                ident[:], is_transpose=True, start=True, stop=True,
            )
        wrow_sb = ph1_pool.tile([1, s], f32r, name="wrow_sb")
        nc.scalar.activation(wrow_sb[:], wrow_ps[:], AF.Exp, scale=0.8)
        wb_sb = ph1_pool.tile([P, s], bf16, name="wb_sb")
        for b in range(ib_n):
            wb_ps = wb_psum.tile([P, nb], f32, name="wb_ps")
            nc.tensor.matmul(
                wb_ps[:], ones_sb[:], wrow_sb[:, b * nb:(b + 1) * nb],
                start=True, stop=True,
            )
            nc.scalar.activation(wb_sb[:, b * nb:(b + 1) * nb], wb_ps[:], AF.Copy)

    for g in range(2, g_n):
        at = adjr_pool.tile([P, GP * s], adt, name="adjr")
        nc.scalar.activation(at[0:1, 0:1], c_sb[0:1, 0:1], AF.Copy)
        nc.scalar.dma_start(
            at[:],
            adjt[g * GP * P:(g + 1) * GP * P, :].rearrange(
                "(p q) i -> p (q i)", q=GP),
        )
        adj_t.append(at)

    # ---- Phase 4: main loop over source-node chunks -----------------------
    if PLAN_B:
        with tc.tile_pool(name="acc", bufs=1, space="PSUM") as acc_psum:
            acc_ps = [acc_psum.tile([P, nb], f32, name=f"acc{sub}")
                      for sub in range(sc_n)]
            for j in range(jc_n):
                mt = m_pool.tile([P, s], bf16, name="mt")
                nc.vector.tensor_scalar_max(mt[:], wb_sb[:], c_sb[:, j:j + 1])
                qq = q_pool.tile([P, s], bf16, name="qq")
                nc.vector.tensor_tensor(
                    qq[:], mt[:],
                    adj_t[j // GP][:, (j % GP) * s:(j % GP + 1) * s],
                    op=ALU.mult,
                )
                for sub in range(sc_n):
                    nc.tensor.matmul(
                        acc_ps[sub][:, 0:PW],
                        qq[:, sub * P:(sub + 1) * P],
                        h2big[:, j * PW:(j + 1) * PW],
                        start=(j == 0), stop=(j == jc_n - 1),
                    )
            for sub in range(sc_n):
                rec_sb = fin_pool.tile([P, 1], f32, name="rec_sb")
                nc.vector.reciprocal(rec_sb[:], acc_ps[sub][:, dout:dout + 1])
                oc_sb = fin_pool.tile([P, dout], f32, name="oc_sb")
                nc.scalar.activation(
                    oc_sb[:], acc_ps[sub][:, 0:dout], AF.Relu,
                    scale=rec_sb[:],
                )
                nc.sync.dma_start(out[sub * P:(sub + 1) * P, :], oc_sb[:])
        return

    # Plan A: weights = [hw] chunk, moving = q; separate row-sum matmuls.
    with tc.tile_pool(name="acc", bufs=1, space="PSUM") as acc_psum, \
         tc.tile_pool(name="tp", bufs=2, space="PSUM") as tp_psum:
        mm_ps = [acc_psum.tile([P, nb], f32, name=f"mm_ps{b}")
                 for b in range(ib_n)]
        rs_ps = [acc_psum.tile([1, nb], f32, name=f"rs_ps{b}")
                 for b in range(ib_n)]
        for j in range(jc_n):
            mt = m_pool.tile([P, s], bf16, name="mt")
            nc.vector.tensor_scalar_max(mt[:], wb_sb[:], c_sb[:, j:j + 1])
            qq = q_pool.tile([P, s], bf16, name="qq")
            nc.vector.tensor_tensor(
                qq[:], mt[:],
                adj_t[j // GP][:, (j % GP) * s:(j % GP + 1) * s],
                op=ALU.mult,
            )
            for b in range(ib_n):
                nc.tensor.matmul(
                    mm_ps[b][:], h2v[:, j, 0:dout], qq[:, b * nb:(b + 1) * nb],
                    start=(j == 0), stop=(j == jc_n - 1),
                )
            for b in range(ib_n):
                nc.tensor.matmul(
                    rs_ps[b][:], ew_sb[:, j:j + 1], qq[:, b * nb:(b + 1) * nb],
                    start=(j == 0), stop=(j == jc_n - 1),
                )

        # ---- Phase 5: normalize, relu, transpose out ----------------------
        rs_sb = ph1_pool.tile([1, s], f32, name="rs_sb")
        for b in range(ib_n):
            nc.scalar.activation(rs_sb[:, b * nb:(b + 1) * nb], rs_ps[b][:],
                                 AF.Copy)
        rsT_ps = tp_psum.tile([P, sc_n], f32, name="rsT_ps", tag="tp")
        for c in range(sc_n):
            nc.tensor.matmul(
                rsT_ps[:, c:c + 1], rs_sb[0:1, c * P:(c + 1) * P],
                ident[:1, :1], is_transpose=True, start=True, stop=True,
            )
        rrT_sb = ph1_pool.tile([P, sc_n], f32, name="rrT_sb")
        nc.vector.reciprocal(rrT_sb[:], rsT_ps[:])

        mo_sb = ph1_pool.tile([P, s], f32r, name="mo_sb")
        for b in range(ib_n):
            nc.scalar.activation(mo_sb[:, b * nb:(b + 1) * nb], mm_ps[b][:],
                                 AF.Copy)
        for c in range(sc_n):
            ot_ps = tp_psum.tile([P, P], f32r, name="ot_ps", tag="tp")
            nc.tensor.matmul(
                ot_ps[:], mo_sb[:, c * P:(c + 1) * P], identr[:],
                is_transpose=True, start=True, stop=True,
            )
            oc_sb = fin_pool.tile([P, dout], f32, name="oc_sb")
            nc.scalar.activation(oc_sb[:], ot_ps[:], AF.Relu,
                                 scale=rrT_sb[:, c:c + 1])
            nc.sync.dma_start(out[c * P:(c + 1) * P, :], oc_sb[:])


def build_nc(n=N, s=S, din=DIN, dout=DOUT):
    from contextlib import ExitStack

    import concourse.bacc as bacc
    import concourse.tile as tile

    nc = bacc.Bacc(
        "TRN2",
        target_bir_lowering=False,
        debug=False,
        num_devices=NCORES,
    )
    with tile.TileContext(nc) as tc, ExitStack() as ctx:
        _emit(nc, tc, ctx, n, s, din, dout)
    nc.compile()
    return nc


def prep_adjt(adj_slab):
    """[s, n] adj row-slab -> [n, s] with GP-chunk interleave.

    DRAM row g*1024 + 8p + q holds adjT row (8g+q)*128 + p, so one DMA per
    super-chunk g reads 8 consecutive rows per partition (16 KB descriptors).
    """
    import ml_dtypes

    dt = ml_dtypes.float8_e4m3 if ADJ_FP8 else ml_dtypes.bfloat16
    adjt = adj_slab.T  # [n, s]
    n, ss = adjt.shape
    P = 128
    adjt = adjt.reshape(n // (GP * P), GP, P, ss).transpose(0, 2, 1, 3)
    return np.ascontiguousarray(adjt.reshape(n, ss).astype(dt))


def make_in_maps(x, adj, W, attn_self, attn_neigh, s=S):
    import ml_dtypes

    att = np.concatenate([attn_self, attn_neigh], axis=1).astype(ml_dtypes.bfloat16)
    wmat = np.ascontiguousarray(W.astype(ml_dtypes.bfloat16))
    wtm = np.ascontiguousarray(W.T.astype(ml_dtypes.bfloat16))
    in_maps = []
    for c in range(NCORES):
        sl = slice(c * s, (c + 1) * s)
        in_maps.append({
            "adjt": prep_adjt(adj[sl, :]),
            "xt": np.ascontiguousarray(x[sl, :].T.astype(ml_dtypes.bfloat16)),
            "wmat": wmat,
            "wt": wtm,
            "att": att,
        })
    return in_maps


def kernel(x, adj, W, attn_self, attn_neigh):
    from concourse.bass_utils import run_bass_kernel_spmd

    x = np.asarray(x, dtype=np.float32)
    adj = np.asarray(adj, dtype=np.float32)
    W = np.asarray(W, dtype=np.float32)
    attn_self = np.asarray(attn_self, dtype=np.float32)
    attn_neigh = np.asarray(attn_neigh, dtype=np.float32)

    nc = build_nc()
    in_maps = make_in_maps(x, adj, W, attn_self, attn_neigh)
    res = run_bass_kernel_spmd(nc, in_maps, list(range(NCORES)))
    return np.concatenate([res.results[c]["out"] for c in range(NCORES)], axis=0)


# revision 24
# speedup vs baseline: 1.1990x; 1.0244x over previous
"""AttentiveGraphConvolution (GAT-style layer) on 8 trn2 NeuronCores.

Math (reference):
    h   = x @ W                       [N, D]
    a_s = h @ attn_self               [N, 1]
    a_n = h @ attn_neigh              [N, 1]
    e   = leaky_relu(a_s + a_n.T, 0.2)
    e   = e + NEG_INF * (1 - adj)
    out = relu(softmax(e, -1) @ h)

Reformulation (exact up to rounding): with exp(leaky(s)) = max(e^s, e^{0.2 s}),

    exp(e_ij) = e^{0.2 a_s_i} * e^{a_n_j} * max(e^{0.8 a_s_i}, e^{-0.8 a_n_j})

The row factor e^{0.2 a_s_i} cancels in the softmax.  With
    wb_i = e^{0.8 a_s_i}      (per output node, broadcast tile)
    c_j  = e^{-0.8 a_n_j}     (per source node, per-partition scalar)
    ew_j = e^{a_n_j}          (row-sum weights)
    hw_j = ew_j * h_j         (pre-scaled h rows)
one DVE op per adjacency chunk produces q_ji = max(wb_i, c_j) * adjT_ji and

    out_i = relu( (sum_j hw_j q_ji) / (sum_j ew_j q_ji) )

Sharding: output rows i across 8 cores; each core streams its adjT slab
[n, s] (bf16, binary-exact).  A single AllGather moves the combined
[hw | ew | a_n] payload (~270 KB/core bf16); everything else is local.
"""

import numpy as np

N = 8192
DIN = 512
DOUT = 128
NCORES = 8
S = N // NCORES     # 1024 output rows per core
GP = 8              # adjacency chunks per super-chunk (one DMA each)
PW = 132            # payload row width: 128 hw | ew | a_n | 2 pad

PLAN_B = True      # True: q-as-weights matmuls (num+den in one pass)
ADJ_FP8 = False     # True: stream adjacency as fp8e4 (binary-exact)


def _emit(nc, tc, ctx, n, s, din, dout):
    from concourse import masks, mybir

    f32 = mybir.dt.float32
    f32r = mybir.dt.float32r
    bf16 = mybir.dt.bfloat16
    fp8 = mybir.dt.float8e4
    adt = fp8 if ADJ_FP8 else bf16
    AF = mybir.ActivationFunctionType
    ALU = mybir.AluOpType

    P = 128
    jc_n = n // P           # 64 source-node chunks
    sc_n = s // P           # 8 chunks in the local row slab
    kc_n = din // P         # 4 contraction chunks for x @ W
    nb = 512                # matmul moving-dim block
    ib_n = s // nb          # 2 i blocks per core
    g_n = jc_n // GP        # 8 adj super-chunks

    adjt = nc.dram_tensor("adjt", [n, s], adt, kind="ExternalInput")
    xt = nc.dram_tensor("xt", [din, s], bf16, kind="ExternalInput")
    wmat = nc.dram_tensor("wmat", [din, dout], bf16, kind="ExternalInput")
    wtr = nc.dram_tensor("wt", [dout, din], bf16, kind="ExternalInput")
    att = nc.dram_tensor("att", [dout, 2], bf16, kind="ExternalInput")
    out = nc.dram_tensor("out", [s, dout], f32, kind="ExternalOutput")

    const_pool = ctx.enter_context(tc.tile_pool(name="const", bufs=1))
    ph1_pool = ctx.enter_context(tc.tile_pool(name="ph1", bufs=1))
    dram_pool = ctx.enter_context(tc.tile_pool(name="dram", bufs=1, space="DRAM"))
    adj_pool = ctx.enter_context(tc.tile_pool(name="adj", bufs=1))
    q_pool = ctx.enter_context(tc.tile_pool(name="q", bufs=4))
    m_pool = ctx.enter_context(tc.tile_pool(name="m", bufs=3))
    adjr_pool = ctx.enter_context(tc.tile_pool(name="adjr", bufs=3))
    fin_pool = ctx.enter_context(tc.tile_pool(name="fin", bufs=2))

    # ---- Phase 1: input loads (xt first: it is on the collective's
    # critical path; adj tiles are emitted after the payload DMA) ----------
    w_sb = []
    x_sb = []
    for k in range(kc_n):
        xt_t = ph1_pool.tile([P, s], bf16, name="x_sb", tag=f"x_sb{k}")
        nc.sync.dma_start(xt_t[:], xt[k * P:(k + 1) * P, :])
        x_sb.append(xt_t)
    for k in range(kc_n):
        wt = ph1_pool.tile([P, dout], bf16, name="w_sb", tag=f"w_sb{k}")
        nc.sync.dma_start(wt[:], wmat[k * P:(k + 1) * P, :])
        w_sb.append(wt)
    att_sb = const_pool.tile([P, 2], bf16, name="att_sb")
    nc.sync.dma_start(att_sb[:], att[:])
    wt_sb = ph1_pool.tile([P, din], bf16, name="wt_sb")
    nc.sync.dma_start(wt_sb[:], wtr[:])

    warm_sb = const_pool.tile([1, 2], f32, name="warm_sb")
    nc.gpsimd.memset(warm_sb[:], 0.0)
    warm_dram = dram_pool.tile([1, 2], f32, name="warm_dram")
    nc.sync.dma_start(warm_dram[:], warm_sb[:])
    warmfull = dram_pool.tile([NCORES, 2], f32, addr_space="Shared",
                              name="warmfull")
    nc.gpsimd.collective_compute(
        "AllGather", ALU.bypass, replica_groups=[list(range(NCORES))],
        ins=[warm_dram.opt()], outs=[warmfull.opt()],
    )

    ident = const_pool.tile([P, P], f32, name="ident")
    masks.make_identity(nc, ident[:])
    identr_t = const_pool.tile([P, P], f32r, name="identr")
    nc.scalar.activation(identr_t[:], ident[:], AF.Copy)
    identr = identr_t[:]
    identb_t = const_pool.tile([2, 2], bf16, name="identb")
    nc.scalar.activation(identb_t[:], ident[:2, :2], AF.Copy)
    identb = identb_t[:]
    ones_f = const_pool.tile([1, P], f32, name="ones_f")
    nc.gpsimd.memset(ones_f[:], 1.0)
    ones_sb = const_pool.tile([1, P], f32r, name="ones_sb")
    nc.scalar.activation(ones_sb[:], ones_f[:], AF.Copy)

    # wsn = (attn.T @ W.T) -> per-k [W | ws | wn] moving tiles so the h
    # matmul also yields a_s/a_n per node (cols 128/129) with no transposes
    with tc.tile_pool(name="ph1ps", bufs=1, space="PSUM") as ph1_psum, \
         tc.tile_pool(name="ph1ps2", bufs=2, space="PSUM") as ph1_psum2:
        wsn_ps = ph1_psum.tile([2, din], f32, name="wsn_ps")
        nc.tensor.matmul(wsn_ps[:], att_sb[:], wt_sb[:],
                         start=True, stop=True)
        wsn_sb = ph1_pool.tile([2, din], bf16, name="wsn_sb")
        nc.scalar.activation(wsn_sb[:], wsn_ps[:], AF.Copy)
        wext = []
        for k in range(kc_n):
            wT_ps = ph1_psum.tile([P, 2], bf16, name="wT_ps")
            nc.tensor.matmul(
                wT_ps[:], wsn_sb[:, k * P:(k + 1) * P], identb[:2, :2],
                is_transpose=True, start=True, stop=True,
            )
            wx = ph1_pool.tile([P, dout + 2], bf16, name="wext", tag=f"wext{k}")
            nc.scalar.activation(wx[:, 0:dout], w_sb[k][:], AF.Copy)
            nc.scalar.activation(wx[:, dout:dout + 2], wT_ps[:], AF.Copy)
            wext.append(wx)

        # ---- Phase 2: combined gather payload [hw | ew | a_n | pad] -------
        hwan_sb = ph1_pool.tile([P, sc_n * PW], bf16, name="hwan_sb")
        nc.gpsimd.memset(hwan_sb[:], 0.0)
        expc_all = ph1_pool.tile([P, sc_n], f32, name="expc_all")
        for c in range(sc_n):
            hn_ps = ph1_psum2.tile([P, dout + 2], f32, name="hn_ps")
            for k in range(kc_n):
                nc.tensor.matmul(
                    hn_ps[:], x_sb[k][:, c * P:(c + 1) * P], wext[k][:],
                    start=(k == 0), stop=(k == kc_n - 1),
                )
            nc.scalar.activation(expc_all[:, c:c + 1],
                                 hn_ps[:, dout + 1:dout + 2],
                                 AF.Exp, scale=1.0)
            nc.scalar.activation(
                hwan_sb[:, c * PW:c * PW + dout], hn_ps[:, 0:dout], AF.Copy,
                scale=expc_all[:, c:c + 1],
            )
            nc.scalar.activation(
                hwan_sb[:, c * PW + dout + 1:c * PW + dout + 2],
                hn_ps[:, dout + 1:dout + 2], AF.Copy)

        hwv = hwan_sb[:].rearrange("p (c d) -> p c d", d=PW)
        nc.scalar.activation(
            hwv[:, :, dout:dout + 1],
            expc_all[:].rearrange("p c -> p c ()"), AF.Copy)

        hwan_dram = dram_pool.tile([s, PW], bf16, name="hwan_dram")
        nc.sync.dma_start(
            hwan_dram[:].rearrange("(p kl) d -> p (kl d)", kl=sc_n), hwan_sb[:])
        hwfull_dram = dram_pool.tile([n, PW], bf16, addr_space="Shared",
                                     name="hwfull")
        nc.gpsimd.collective_compute(
            "AllGather", ALU.bypass, replica_groups=[list(range(NCORES))],
            ins=[hwan_dram.opt()], outs=[hwfull_dram.opt()],
        )

        # ---- adjacency prefetch, all gated: g0/g1 on av_sb (t~15us, after
        # xt is consumed), the rest on c_sb (post-readback) so the payload
        # DMA, collective and readback never queue behind adj traffic ------
        adj_t = []
        for g in range(2):
            at = adj_pool.tile([P, GP * s], adt, name="adj_t", tag=f"adj{g}")
            nc.scalar.activation(at[0:1, 0:1], wext[3][0:1, 0:1], AF.Copy)
            nc.scalar.dma_start(
                at[:],
                adjt[g * GP * P:(g + 1) * GP * P, :].rearrange(
                    "(p q) i -> p (q i)", q=GP),
            )
            adj_t.append(at)


    # ---- Phase 3: readback of the gathered payload ------------------------
    # h2big[p, j*PW + d] = hwfull[node 128j + p, d]
    h2big = ph1_pool.tile([P, jc_n * PW], bf16, name="h2big")
    for cblk in range(NCORES):
        nc.sync.dma_start(
            h2big[:, cblk * sc_n * PW:(cblk + 1) * sc_n * PW],
            hwfull_dram[cblk * s:(cblk + 1) * s, :].rearrange(
                "(p kl) d -> p (kl d)", kl=sc_n),
        )
    h2v = h2big[:].rearrange("p (j d) -> p j d", d=PW)
    c_sb = ph1_pool.tile([P, jc_n], f32, name="c_sb")
    nc.scalar.activation(
        c_sb[:].rearrange("p j -> p j ()"), h2v[:, :, dout + 1:dout + 2],
        AF.Exp, scale=-0.8,
    )
    ew_sb = ph1_pool.tile([P, jc_n], bf16, name="ew_sb")
    nc.scalar.activation(
        ew_sb[:].rearrange("p j -> p j ()"), h2v[:, :, dout:dout + 1], AF.Copy)

    # wb[p, i] = e^{0.8 a_s_i}: a_s row recomputed from x (runs in the
    # gather window on the otherwise idle PE), then exp + broadcast
    with tc.tile_pool(name="wbps", bufs=2, space="PSUM") as wb_psum:
        wrow_sb = ph1_pool.tile([1, s], f32r, name="wrow_sb")
        wb_sb = ph1_pool.tile([P, s], bf16, name="wb_sb")
        for b in range(ib_n):
            av_ps = wb_psum.tile([2, nb], f32, name="av_ps")
            for k in range(kc_n):
                nc.tensor.matmul(
                    av_ps[:], wext[k][:, dout:dout + 2],
                    x_sb[k][:, b * nb:(b + 1) * nb],
                    start=(k == 0), stop=(k == kc_n - 1),
                )
            nc.scalar.activation(wrow_sb[:, b * nb:(b + 1) * nb],
                                 av_ps[0:1, :], AF.Exp, scale=0.8)
            wb_ps = wb_psum.tile([P, nb], f32, name="wb_ps")
            nc.tensor.matmul(
                wb_ps[:], ones_sb[:], wrow_sb[:, b * nb:(b + 1) * nb],
                start=True, stop=True,
            )
            nc.scalar.activation(wb_sb[:, b * nb:(b + 1) * nb], wb_ps[:], AF.Copy)

    for g in range(2, g_n):
        at = adjr_pool.tile([P, GP * s], adt, name="adjr")
        nc.scalar.activation(at[0:1, 0:1], c_sb[0:1, 0:1], AF.Copy)
        nc.scalar.dma_start(
            at[:],
            adjt[g * GP * P:(g + 1) * GP * P, :].rearrange(
                "(p q) i -> p (q i)", q=GP),
        )
        adj_t.append(at)

    # ---- Phase 4: main loop over source-node chunks -----------------------
    if PLAN_B:
        with tc.tile_pool(name="acc", bufs=1, space="PSUM") as acc_psum:
            acc_ps = [acc_psum.tile([P, nb], f32, name=f"acc{sub}")
                      for sub in range(sc_n)]
            for j in range(jc_n):
                mt = m_pool.tile([P, s], bf16, name="mt")
                nc.vector.tensor_scalar_max(mt[:], wb_sb[:], c_sb[:, j:j + 1])
                qq = q_pool.tile([P, s], bf16, name="qq")
                nc.vector.tensor_tensor(
                    qq[:], mt[:],
                    adj_t[j // GP][:, (j % GP) * s:(j % GP + 1) * s],
                    op=ALU.mult,
                )
                for sub in range(sc_n):
                    nc.tensor.matmul(
                        acc_ps[sub][:, 0:PW],
                        qq[:, sub * P:(sub + 1) * P],
                        h2big[:, j * PW:(j + 1) * PW],
                        start=(j == 0), stop=(j == jc_n - 1),
                    )
            for sub in range(sc_n):
                rec_sb = fin_pool.tile([P, 1], f32, name="rec_sb")
                nc.vector.reciprocal(rec_sb[:], acc_ps[sub][:, dout:dout + 1])
                oc_sb = fin_pool.tile([P, dout], f32, name="oc_sb")
                nc.scalar.activation(
                    oc_sb[:], acc_ps[sub][:, 0:dout], AF.Relu,
                    scale=rec_sb[:],
                )
                nc.sync.dma_start(out[sub * P:(sub + 1) * P, :], oc_sb[:])
        return

    # Plan A: weights = [hw] chunk, moving = q; separate row-sum matmuls.
    with tc.tile_pool(name="acc", bufs=1, space="PSUM") as acc_psum, \
         tc.tile_pool(name="tp", bufs=2, space="PSUM") as tp_psum:
        mm_ps = [acc_psum.tile([P, nb], f32, name=f"mm_ps{b}")
                 for b in range(ib_n)]
        rs_ps = [acc_psum.tile([1, nb], f32, name=f"rs_ps{b}")
                 for b in range(ib_n)]
        for j in range(jc_n):
            mt = m_pool.tile([P, s], bf16, name="mt")
            nc.vector.tensor_scalar_max(mt[:], wb_sb[:], c_sb[:, j:j + 1])
            qq = q_pool.tile([P, s], bf16, name="qq")
            nc.vector.tensor_tensor(
                qq[:], mt[:],
                adj_t[j // GP][:, (j % GP) * s:(j % GP + 1) * s],
                op=ALU.mult,
            )
            for b in range(ib_n):
                nc.tensor.matmul(
                    mm_ps[b][:], h2v[:, j, 0:dout], qq[:, b * nb:(b + 1) * nb],
                    start=(j == 0), stop=(j == jc_n - 1),
                )
            for b in range(ib_n):
                nc.tensor.matmul(
                    rs_ps[b][:], ew_sb[:, j:j + 1], qq[:, b * nb:(b + 1) * nb],
                    start=(j == 0), stop=(j == jc_n - 1),
                )

        # ---- Phase 5: normalize, relu, transpose out ----------------------
        rs_sb = ph1_pool.tile([1, s], f32, name="rs_sb")
        for b in range(ib_n):
            nc.scalar.activation(rs_sb[:, b * nb:(b + 1) * nb], rs_ps[b][:],
                                 AF.Copy)
        rsT_ps = tp_psum.tile([P, sc_n], f32, name="rsT_ps", tag="tp")
        for c in range(sc_n):
            nc.tensor.matmul(
                rsT_ps[:, c:c + 1], rs_sb[0:1, c * P:(c + 1) * P],
                ident[:1, :1], is_transpose=True, start=True, stop=True,
            )
        rrT_sb = ph1_pool.tile([P, sc_n], f32, name="rrT_sb")
        nc.vector.reciprocal(rrT_sb[:], rsT_ps[:])

        mo_sb = ph1_pool.tile([P, s], f32r, name="mo_sb")
        for b in range(ib_n):
            nc.scalar.activation(mo_sb[:, b * nb:(b + 1) * nb], mm_ps[b][:],
                                 AF.Copy)
        for c in range(sc_n):
            ot_ps = tp_psum.tile([P, P], f32r, name="ot_ps", tag="tp")
            nc.tensor.matmul(
                ot_ps[:], mo_sb[:, c * P:(c + 1) * P], identr[:],
                is_transpose=True, start=True, stop=True,
            )
            oc_sb = fin_pool.tile([P, dout], f32, name="oc_sb")
            nc.scalar.activation(oc_sb[:], ot_ps[:], AF.Relu,
                                 scale=rrT_sb[:, c:c + 1])
            nc.sync.dma_start(out[c * P:(c + 1) * P, :], oc_sb[:])


def build_nc(n=N, s=S, din=DIN, dout=DOUT):
    from contextlib import ExitStack

    import concourse.bacc as bacc
    import concourse.tile as tile

    nc = bacc.Bacc(
        "TRN2",
        target_bir_lowering=False,
        debug=False,
        num_devices=NCORES,
    )
    with tile.TileContext(nc) as tc, ExitStack() as ctx:
        _emit(nc, tc, ctx, n, s, din, dout)
    nc.compile()
    return nc


def prep_adjt(adj_slab):
    """[s, n] adj row-slab -> [n, s] with GP-chunk interleave.

    DRAM row g*1024 + 8p + q holds adjT row (8g+q)*128 + p, so one DMA per
    super-chunk g reads 8 consecutive rows per partition (16 KB descriptors).
    """
    import ml_dtypes

    dt = ml_dtypes.float8_e4m3 if ADJ_FP8 else ml_dtypes.bfloat16
    adjt = adj_slab.T  # [n, s]
    n, ss = adjt.shape
    P = 128
    adjt = adjt.reshape(n // (GP * P), GP, P, ss).transpose(0, 2, 1, 3)
    return np.ascontiguousarray(adjt.reshape(n, ss).astype(dt))


def make_in_maps(x, adj, W, attn_self, attn_neigh, s=S):
    import ml_dtypes

    att = np.concatenate([attn_self, attn_neigh], axis=1).astype(ml_dtypes.bfloat16)
    wmat = np.ascontiguousarray(W.astype(ml_dtypes.bfloat16))
    wtm = np.ascontiguousarray(W.T.astype(ml_dtypes.bfloat16))
    in_maps = []
    for c in range(NCORES):
        sl = slice(c * s, (c + 1) * s)
        in_maps.append({
            "adjt": prep_adjt(adj[sl, :]),
            "xt": np.ascontiguousarray(x[sl, :].T.astype(ml_dtypes.bfloat16)),
            "wmat": wmat,
            "wt": wtm,
            "att": att,
        })
    return in_maps


def kernel(x, adj, W, attn_self, attn_neigh):
    from concourse.bass_utils import run_bass_kernel_spmd

    x = np.asarray(x, dtype=np.float32)
    adj = np.asarray(adj, dtype=np.float32)
    W = np.asarray(W, dtype=np.float32)
    attn_self = np.asarray(attn_self, dtype=np.float32)
    attn_neigh = np.asarray(attn_neigh, dtype=np.float32)

    nc = build_nc()
    in_maps = make_in_maps(x, adj, W, attn_self, attn_neigh)
    res = run_bass_kernel_spmd(nc, in_maps, list(range(NCORES)))
    return np.concatenate([res.results[c]["out"] for c in range(NCORES)], axis=0)
